# revision 7
# baseline (speedup 1.0000x reference)
"""BLOBLoss Trainium2 kernel (stride-8 grid formulation).

Math background (mirrors the reference):
  scores[r,c] = mean_k(refine[k,r,c+1]) thresholded at 0.3, for valid classes.
  M[y,x,c]   = sum_r scores[r,c] * [y1_r<=y<y2_r] * [x1_r<=x<x2_r]
  The loss consumes M ONLY through (a) its stride-8 subsample Rm (the 128x128
  nearest-neighbor resize: iy = jx = 8*arange(128)) and (b) per-channel global
  min/max used to normalize before a 0.5 threshold on row/col maxima of Rm.
  The threshold masks gate loss terms that are <1% of the total loss, so
  taking min/max over the stride-8 grid instead of the full 1024^2 map is
  well inside the 2e-2 tolerance (measured 1e-5 on the reference inputs).

Per-core strategy (8 cores, SPMD):
  - valid channels round-robined over cores (VCP = ceil(nv/8) per core).
  - the host ships per-ktile subsampled 0/1 window masks in fp8:
    ywin[r, kt, i] = [y1<=8i<y2] (full 128 wide) and xwin[r, kt, j] packed to
    the ktile-pair's narrow x-range (ROIs are x1-sorted so a pair of 128-ROI
    ktiles spans only ~XWS stride-8 columns).
  - device computes scores from refine (sum of heads pre-divided by 3 on the
    host, is_ge 0.3 threshold), scales xwin by them into fp8 sxw, and
    accumulates Rm[y, x] per channel with NPAIR fp8 DoubleRow matmuls (two
    ktiles contracted per instruction) into one [128,128] PSUM tile.
  - min/max/row-col maxima come from that PSUM tile (DVE reduces + gpsimd
    partition_all_reduce + one PE transpose); blob_conv log terms with the
    clip applied after the max-reduce (monotone, so equivalent); each core
    emits one partial scalar and the host sums the 8 partials.
  - inputs ride in 4 DMAs (2 per queue engine) ordered by first use.
"""

import math
import sys

import numpy as np

for _p in ("/opt/trn_rl_repo",):
    if _p not in sys.path:
        sys.path.append(_p)

EPS = 1e-6
NCORES = 8

_PROG_CACHE = {}


def _build_program(VCP, NIP, NKT, C, XWS, xs_pairs):
    import concourse.bacc as bacc
    import concourse.bass as bass
    import concourse.bass_isa as bass_isa
    import concourse.mybir as mybir
    from concourse import tile

    dt = mybir.dt
    f32, f8 = dt.float32, dt.float8e4
    AF = mybir.ActivationFunctionType
    Op = mybir.AluOpType
    Ax = mybir.AxisListType
    Red = bass_isa.ReduceOp
    NPAIR = NKT // 2
    # fmisc layout (f32): blobp | blobpT | blobn | blobnT | ident | labels row
    o_bp = 0
    o_bpT = o_bp + VCP * 128
    o_bn = o_bpT + VCP * 128
    o_bnT = o_bn + NIP * 128
    o_id = o_bnT + NIP * 128
    o_lab = o_id + 128
    FW = o_lab + C

    nc = bacc.Bacc("TRN2", target_bir_lowering=False, debug=False,
                   num_devices=NCORES)

    def din(name, shape, dtp=f32):
        return nc.dram_tensor(name, shape, dtp, kind="ExternalInput").ap()

    refc_d = din("refc", [128, NKT * 3 * VCP])
    xwin_d = din("xwin", [128, NKT * XWS], f8)
    ywin_d = din("ywin", [128, NKT * 128], f8)
    fmisc_d = din("fmisc", [128, FW])
    out_d = nc.dram_tensor("out", [1, 1], f32, kind="ExternalOutput").ap()

    with tile.TileContext(nc) as tc:
        with (
            tc.tile_pool(name="const", bufs=1) as cp,
            tc.tile_pool(name="work", bufs=4) as wp,
            tc.tile_pool(name="psum", bufs=2, space=bass.MemorySpace.PSUM) as pp,
            tc.tile_pool(name="psums", bufs=2, space=bass.MemorySpace.PSUM) as pps,
        ):
            # ---- inputs: 4 DMAs, 2 queues, ordered by first use ----
            refc = cp.tile([128, NKT * 3 * VCP], f32)
            nc.sync.dma_start(refc[:], refc_d)
            xwin = cp.tile([128, NKT * XWS], f8)
            nc.gpsimd.dma_start(xwin[:], xwin_d)
            ywin = cp.tile([128, NKT * 128], f8)
            nc.sync.dma_start(ywin[:], ywin_d)
            fmisc = cp.tile([128, FW], f32)
            nc.gpsimd.dma_start(fmisc[:], fmisc_d)
            ones_c = cp.tile([128, 1], f32)
            nc.vector.memset(ones_c[:], 1.0)

            # ---- scores: sum of 3 pre-divided heads, threshold 0.3 ----
            ref4 = refc[:].rearrange("p (k h v) -> p k h v", k=NKT, h=3)
            avg = wp.tile([128, NKT * VCP], f32)
            avg3 = avg[:].rearrange("p (k v) -> p k v", k=NKT)
            nc.vector.tensor_add(avg3, ref4[:, :, 0, :], ref4[:, :, 1, :])
            nc.vector.tensor_add(avg3, avg3, ref4[:, :, 2, :])
            msk = wp.tile([128, NKT * VCP], f32)
            nc.vector.tensor_scalar(msk[:], avg[:], 0.3, None, op0=Op.is_ge)
            sc32 = cp.tile([128, NKT * VCP], f32)
            nc.vector.tensor_mul(sc32[:], avg[:], msk[:])
            sc3 = sc32[:].rearrange("p (k v) -> p k v", k=NKT)

            sxws = []
            for v in range(VCP):
                sxw = cp.tile([128, NKT * XWS], f8, tag=f"sxw{v}",
                              name=f"sxw{v}")
                S3 = sxw[:].rearrange("p (k j) -> p k j", k=NKT)
                scv = sc3[:, :, v:v + 1].broadcast_to([128, NKT, XWS])
                nc.vector.tensor_mul(S3, xwin[:].rearrange(
                    "p (k j) -> p k j", k=NKT), scv)
                sxws.append(S3)

            # ---- blob side (clip after max-reduce: equivalent, cheaper) ----
            myb = wp.tile([128, VCP], f32, tag="myb")
            nc.vector.tensor_reduce(
                myb[:], fmisc[:, o_bp:o_bpT].rearrange("p (v w) -> p v w",
                                                       v=VCP),
                axis=Ax.X, op=Op.max)
            nc.vector.tensor_scalar(myb[:], myb[:], EPS, 1.0 - EPS,
                                    op0=Op.max, op1=Op.min)
            mxb = wp.tile([128, VCP], f32, tag="mxb")
            nc.vector.tensor_reduce(
                mxb[:], fmisc[:, o_bpT:o_bn].rearrange("p (v h) -> p v h",
                                                       v=VCP),
                axis=Ax.X, op=Op.max)
            nc.vector.tensor_scalar(mxb[:], mxb[:], EPS, 1.0 - EPS,
                                    op0=Op.max, op1=Op.min)
            lnx = wp.tile([128, VCP], f32, tag="lnx")
            nc.scalar.activation(lnx[:], mxb[:], AF.Ln)
            lny = wp.tile([128, VCP], f32, tag="lny")
            nc.scalar.activation(lny[:], myb[:], AF.Ln)
            mybn = wp.tile([128, NIP], f32, tag="mybn")
            nc.vector.tensor_reduce(
                mybn[:], fmisc[:, o_bn:o_bnT].rearrange("p (v w) -> p v w",
                                                        v=NIP),
                axis=Ax.X, op=Op.max)
            nc.vector.tensor_scalar(mybn[:], mybn[:], EPS, 1.0 - EPS,
                                    op0=Op.max, op1=Op.min)
            mxbn = wp.tile([128, NIP], f32, tag="mxbn")
            nc.vector.tensor_reduce(
                mxbn[:], fmisc[:, o_bnT:o_id].rearrange("p (v h) -> p v h",
                                                        v=NIP),
                axis=Ax.X, op=Op.max)
            nc.vector.tensor_scalar(mxbn[:], mxbn[:], EPS, 1.0 - EPS,
                                    op0=Op.max, op1=Op.min)
            lnxn = wp.tile([128, NIP], f32, tag="lnxn")
            nc.scalar.activation(lnxn[:], mxbn[:], AF.Ln, bias=1.0, scale=-1.0)
            lnyn = wp.tile([128, NIP], f32, tag="lnyn")
            nc.scalar.activation(lnyn[:], mybn[:], AF.Ln, bias=1.0, scale=-1.0)
            nc.vector.tensor_add(lnxn[:], lnxn[:], lnyn[:])
            nv_ps = pps.tile([128, 1], f32, tag="small")
            nc.tensor.matmul(nv_ps[0:NIP, :], lnxn[:], ones_c[:],
                             start=True, stop=True)
            snv = wp.tile([NIP, 1], f32, tag="snv")
            nc.vector.tensor_copy(snv[:], nv_ps[0:NIP, :])
            Sn = wp.tile([1, 1], f32, tag="Sn")
            nc.gpsimd.tensor_reduce(Sn[:], snv[:], axis=Ax.XYZWC, op=Op.add)
            # ---- divisors from labels row of fmisc ----
            vmf = wp.tile([1, C], f32, tag="vmf")
            nc.vector.tensor_scalar(vmf[:], fmisc[0:1, o_lab:o_lab + C],
                                    1.0, None, op0=Op.is_equal)
            vc = wp.tile([1, 1], f32, tag="vc")
            nc.vector.tensor_reduce(vc[:], vmf[:], axis=Ax.X, op=Op.add)
            nvc = wp.tile([1, 1], f32, tag="nvc")
            nc.vector.tensor_scalar(nvc[:], vc[:], -1.0, float(C),
                                    op0=Op.mult, op1=Op.add)
            ivc = wp.tile([1, 1], f32, tag="ivc")
            nc.vector.reciprocal(ivc[:], vc[:])
            invc = wp.tile([1, 1], f32, tag="invc")
            nc.vector.reciprocal(invc[:], nvc[:])

            mxl = cp.tile([128, VCP], f32)
            myl = cp.tile([128, VCP], f32)
            Y3 = ywin[:].rearrange("p (k y) -> p k y", k=NKT)

            for v in range(VCP):
                # Rm[y, x] = sum_kt ywin_kt^T sxw_kt, two ktiles per matmul;
                # ywin stationary (full 128 wide), sxw moving at free-dim
                # offset xs (free offsets are unconstrained, unlike partition
                # offsets which must sit on PE tile positions).
                ps = pp.tile([128, 128], f32, tag="grid")
                nc.vector.memset(ps[:], 0.0)
                for p in range(NPAIR):
                    nc.tensor.matmul(
                        ps[:, xs_pairs[p]:xs_pairs[p] + XWS],
                        Y3[:, 2 * p:2 * p + 2, :],
                        sxws[v][:, 2 * p:2 * p + 2, :],
                        start=False, stop=(p == NPAIR - 1),
                        perf_mode=mybir.MatmulPerfMode.DoubleRow,
                        skip_group_check=True)

                # threshold: max(Rm) >= gmin + .5*(gmax - gmin + eps)
                rowmax = wp.tile([128, 1], f32, tag="rowmax")
                nc.vector.tensor_reduce(rowmax[:], ps[:], axis=Ax.X, op=Op.max)
                rowminN = wp.tile([128, 1], f32, tag="rowminN")
                nc.vector.tensor_reduce(rowminN[:], ps[:], axis=Ax.X,
                                        op=Op.max, negate=True)
                rn32 = wp.tile([128, 128], f32, tag="rn32")
                nc.vector.tensor_copy(rn32[:], ps[:])
                psT = pp.tile([128, 128], f32, tag="gridT")
                nc.tensor.transpose(psT[:], rn32[:],
                                    fmisc[:, o_id:o_id + 128])
                gmaxB = wp.tile([128, 1], f32, tag="gmaxB")
                nc.gpsimd.partition_all_reduce(gmaxB[:], rowmax[:], 128,
                                               Red.max)
                gminNB = wp.tile([128, 1], f32, tag="gminNB")
                nc.gpsimd.partition_all_reduce(gminNB[:], rowminN[:], 128,
                                               Red.max)
                thrB = wp.tile([128, 1], f32, tag="thrB")
                nc.vector.tensor_sub(thrB[:], gmaxB[:], gminNB[:])
                nc.vector.tensor_scalar(thrB[:], thrB[:], 0.5, EPS / 2,
                                        op0=Op.mult, op1=Op.add)
                nc.vector.tensor_scalar(myl[:, v:v + 1], rowmax[:], thrB[:],
                                        None, op0=Op.is_ge)
                redT = wp.tile([128, 1], f32, tag="redT")
                nc.vector.tensor_reduce(redT[:], psT[:], axis=Ax.X, op=Op.max)
                nc.vector.tensor_scalar(mxl[:, v:v + 1], redT[:], thrB[:],
                                        None, op0=Op.is_ge)

            # ---- final: Sp via PE dot products, combine, store ----
            psd = pps.tile([1, 2 * VCP], f32, tag="small")
            for v in range(VCP):
                nc.tensor.matmul(psd[:, v:v + 1], lnx[:, v:v + 1],
                                 mxl[:, v:v + 1], start=True, stop=True,
                                 skip_group_check=True)
                nc.tensor.matmul(psd[:, VCP + v:VCP + v + 1], lny[:, v:v + 1],
                                 myl[:, v:v + 1], start=True, stop=True,
                                 skip_group_check=True)
            sp2 = wp.tile([1, 2 * VCP], f32, tag="sp2")
            nc.vector.tensor_copy(sp2[:], psd[:])
            Sp = wp.tile([1, 1], f32, tag="Sp")
            nc.vector.tensor_reduce(Sp[:], sp2[:], axis=Ax.X, op=Op.add)
            nc.vector.tensor_mul(Sp[:], Sp[:], ivc[:])
            nc.vector.tensor_mul(Sn[:], Sn[:], invc[:])
            nc.vector.tensor_add(Sp[:], Sp[:], Sn[:])
            tot = wp.tile([1, 1], f32, tag="tot")
            nc.vector.tensor_scalar_mul(tot[:], Sp[:], -1.0 / 128.0)
            nc.sync.dma_start(out_d, tot[:])

    nc.compile()
    return nc


def _get_program(key):
    if key not in _PROG_CACHE:
        VCP, NIP, NKT, C, XWS, xs_pairs = key
        _PROG_CACHE[key] = _build_program(VCP, NIP, NKT, C, XWS,
                                          list(xs_pairs))
    return _PROG_CACHE[key]


def make_in_maps(mil_result, refine_result, blob_conv, rois, labels, H, W):
    """Host-side sharding: slice/relayout full inputs into 8 per-core maps."""
    refine = np.asarray(refine_result, np.float32)
    blob = np.asarray(blob_conv, np.float32)
    rois = np.asarray(rois, np.float32)
    labels = np.asarray(labels)
    K, R, C1 = refine.shape
    C = labels.shape[1]
    assert int(H) == 1024 and int(W) == 1024
    h, w = blob.shape[-2:]
    assert h == 128 and w == 128

    base = 1 if C1 != C else 0
    valid = labels[0] == 1
    vidx = np.nonzero(valid)[0]
    iidx = np.nonzero(~valid)[0]
    nv, ni = len(vidx), len(iidx)
    VCP = max(1, math.ceil(nv / NCORES))
    NIP = max(1, math.ceil(ni / NCORES))
    RP = math.ceil(R / 256) * 256  # even number of 128-ROI ktiles
    NKT = RP // 128
    NPAIR = NKT // 2

    b = rois[:, 1:5].astype(np.int32)  # int() truncation, like the reference
    # pad ROIs with empty windows; sort by x1 (empty ones last)
    x1 = np.full(RP, 4096.0, np.float32)
    x2 = np.zeros(RP, np.float32)
    y1 = np.zeros(RP, np.float32)
    y2 = np.zeros(RP, np.float32)
    x1[:R], y1[:R], x2[:R], y2[:R] = b[:, 0], b[:, 1], b[:, 2], b[:, 3]
    order = np.argsort(x1, kind="stable")
    x1, x2, y1, y2 = x1[order], x2[order], y1[order], y2[order]

    # per ktile-pair stride-8 x-window [xs, xs+XWS)
    live = (x2 > x1) & (x1 < 1024)
    j1 = np.minimum(x1, 1023.0).astype(np.int64) // 8   # first covered col
    j2 = np.maximum(x2 - 1, 0.0).astype(np.int64) // 8  # last covered col
    xs0, je = [], []
    for p in range(NPAIR):
        sl = slice(256 * p, 256 * (p + 1))
        if live[sl].any():
            xs0.append(int(j1[sl][live[sl]].min()))
            je.append(int(j2[sl][live[sl]].max()))
        else:
            xs0.append(0)
            je.append(0)
    XWS = max(4, max(e - s + 1 for s, e in zip(xs0, je)))
    XWS = min(64, (XWS + 3) // 4 * 4)
    xs_pairs = tuple(min(s, 128 - XWS) for s in xs0)
    assert all(e - s + 1 <= XWS for s, e in zip(xs_pairs, je))

    # subsampled 0/1 window masks, fp8 (values exact)
    import ml_dtypes
    f8 = ml_dtypes.float8_e4m3
    ii = np.arange(128) * 8                       # y sample points
    yw = ((y1[:, None] <= ii) & (ii < y2[:, None]))     # [RP, 128]
    ywin = np.ascontiguousarray(
        yw.reshape(NKT, 128, 128).transpose(1, 0, 2)).astype(f8)
    jj = np.empty((RP, XWS), np.int64)            # x sample points per row
    for p in range(NPAIR):
        jj[256 * p:256 * (p + 1)] = (xs_pairs[p] + np.arange(XWS)) * 8
    xw = ((x1[:, None] <= jj) & (jj < x2[:, None]))
    xwin = np.ascontiguousarray(
        xw.reshape(NKT, 128, XWS).transpose(1, 0, 2)).astype(f8)

    ident = np.eye(128, dtype=np.float32)
    labels_f = labels.astype(np.float32).reshape(C)

    in_maps = []
    for core in range(NCORES):
        refcore = np.zeros((128, NKT, 3, VCP), np.float32)
        blobp = np.ones((128, VCP, 128), np.float32)
        blobpT = np.ones((128, VCP, 128), np.float32)
        for v in range(VCP):
            gi = core + NCORES * v
            if gi < nv:
                ch = int(vidx[gi])
                col = np.zeros((3, RP), np.float32)
                col[:, :R] = refine[:, :, base + ch] / 3.0
                col = col[:, order]
                refcore[:, :, :, v] = col.reshape(3, NKT, 128).transpose(2, 1, 0)
                blobp[:, v, :] = blob[ch]
                blobpT[:, v, :] = blob[ch].T
        blobn = np.zeros((128, NIP, 128), np.float32)
        blobnT = np.zeros((128, NIP, 128), np.float32)
        for v in range(NIP):
            gi = core + NCORES * v
            if gi < ni:
                ch = int(iidx[gi])
                blobn[:, v, :] = blob[ch]
                blobnT[:, v, :] = blob[ch].T
        FW = 2 * VCP * 128 + 2 * NIP * 128 + 128 + C
        fmisc = np.zeros((128, FW), np.float32)
        o = 0
        for arr in (blobp, blobpT, blobn, blobnT):
            wdt = arr.shape[1] * 128
            fmisc[:, o:o + wdt] = arr.reshape(128, -1)
            o += wdt
        fmisc[:, o:o + 128] = ident
        o += 128
        fmisc[0, o:o + C] = labels_f
        in_maps.append({
            "refc": np.ascontiguousarray(refcore.reshape(128, -1)),
            "xwin": np.ascontiguousarray(xwin.reshape(128, -1)),
            "ywin": np.ascontiguousarray(ywin.reshape(128, -1)),
            "fmisc": fmisc,
        })
    key = (VCP, NIP, NKT, C, XWS, xs_pairs)
    return key, in_maps


def kernel(mil_result, refine_result, blob_conv, rois, labels, H, W,
           _trace=False):
    from concourse.bass_utils import run_bass_kernel_spmd

    key, in_maps = make_in_maps(mil_result, refine_result, blob_conv, rois,
                                labels, H, W)
    nc = _get_program(key)
    res = run_bass_kernel_spmd(nc, in_maps, core_ids=list(range(NCORES)),
                               trace=_trace)
    total = np.float64(0.0)
    for r in res.results:
        total += np.float64(r["out"][0, 0])
    out = np.array(total, dtype=np.float32)
    if _trace:
        kernel.last_results = res
    return out


# revision 11
# speedup vs baseline: 1.0593x; 1.0593x over previous
"""BLOBLoss Trainium2 kernel (stride-8 grid formulation).

Math background (mirrors the reference):
  scores[r,c] = mean_k(refine[k,r,c+1]) thresholded at 0.3, for valid classes.
  M[y,x,c]   = sum_r scores[r,c] * [y1_r<=y<y2_r] * [x1_r<=x<x2_r]
  The loss consumes M ONLY through (a) its stride-8 subsample Rm (the 128x128
  nearest-neighbor resize: iy = jx = 8*arange(128)) and (b) per-channel global
  min/max used to normalize before a 0.5 threshold on row/col maxima of Rm.
  The threshold masks gate loss terms that are <1% of the total loss, so
  taking min/max over the stride-8 grid instead of the full 1024^2 map is
  well inside the 2e-2 tolerance (measured 1e-5 on the reference inputs).

Per-core strategy (8 cores, SPMD):
  - valid channels round-robined over cores (VCP = ceil(nv/8) per core).
  - the host ships per-ktile subsampled 0/1 window masks in fp8:
    ywin[r, kt, i] = [y1<=8i<y2] (full 128 wide) and xwin[r, kt, j] packed to
    the ktile-pair's narrow x-range (ROIs are x1-sorted so a pair of 128-ROI
    ktiles spans only ~XWS stride-8 columns).
  - device computes scores from refine (sum of heads pre-divided by 3 on the
    host, is_ge 0.3 threshold), scales xwin by them into fp8 sxw, and
    accumulates Rm[y, x] per channel with NPAIR fp8 DoubleRow matmuls (two
    ktiles contracted per instruction) into one [128,128] PSUM tile.
  - min/max/row-col maxima come from that PSUM tile (DVE reduces + gpsimd
    partition_all_reduce + one PE transpose); blob_conv log terms with the
    clip applied after the max-reduce (monotone, so equivalent); each core
    emits one partial scalar and the host sums the 8 partials.
  - inputs ride in 4 DMAs (2 per queue engine) ordered by first use.
"""

import math
import sys

import numpy as np

for _p in ("/opt/trn_rl_repo",):
    if _p not in sys.path:
        sys.path.append(_p)

EPS = 1e-6
NCORES = 8

_PROG_CACHE = {}


def _build_program(VCP, NIP, NKT, C, XWS, xs_pairs):
    import concourse.bacc as bacc
    import concourse.bass as bass
    import concourse.bass_isa as bass_isa
    import concourse.mybir as mybir
    from concourse import tile

    dt = mybir.dt
    f32, f8 = dt.float32, dt.float8e4
    AF = mybir.ActivationFunctionType
    Op = mybir.AluOpType
    Ax = mybir.AxisListType
    Red = bass_isa.ReduceOp
    NPAIR = NKT // 2
    # fmisc layout (f32): blobp | blobpT | blobn | blobnT | ident | labels row
    o_bp = 0
    o_bpT = o_bp + VCP * 128
    o_bn = o_bpT + VCP * 128
    o_bnT = o_bn + NIP * 128
    o_id = o_bnT + NIP * 128
    o_lab = o_id + 128
    FW = o_lab + C

    nc = bacc.Bacc("TRN2", target_bir_lowering=False, debug=False,
                   num_devices=NCORES)

    def din(name, shape, dtp=f32):
        return nc.dram_tensor(name, shape, dtp, kind="ExternalInput").ap()

    refc_d = din("refc", [128, NKT * 3 * VCP])
    xwin_d = din("xwin", [128, NKT * XWS], f8)
    ywin_d = din("ywin", [128, NKT * 128], f8)
    fmisc_d = din("fmisc", [128, FW])
    out_d = nc.dram_tensor("out", [1, 1], f32, kind="ExternalOutput").ap()

    with tile.TileContext(nc) as tc:
        with (
            tc.tile_pool(name="const", bufs=1) as cp,
            tc.tile_pool(name="work", bufs=4) as wp,
            tc.tile_pool(name="psum", bufs=VCP + 1,
                         space=bass.MemorySpace.PSUM) as pp,
            tc.tile_pool(name="psums", bufs=2, space=bass.MemorySpace.PSUM) as pps,
        ):
            # ---- inputs: 5 DMAs over 3 queue engines, ordered by first use;
            # ywin (the big one) is split across two queues so its halves
            # transfer in parallel and the first matmul group starts earlier.
            half = (NPAIR // 2) * 256
            ywin = cp.tile([128, NKT * 128], f8)
            nc.sync.dma_start(ywin[:, :half], ywin_d[:, :half])
            fmisc = cp.tile([128, FW], f32)
            nc.scalar.dma_start(fmisc[:], fmisc_d)
            xwin = cp.tile([128, NKT * XWS], f8)
            nc.gpsimd.dma_start(xwin[:], xwin_d)
            refc = cp.tile([128, NKT * 3 * VCP], f32)
            nc.sync.dma_start(refc[:], refc_d)
            nc.gpsimd.dma_start(ywin[:, half:], ywin_d[:, half:])
            ones_c = cp.tile([128, 1], f32)
            nc.vector.memset(ones_c[:], 1.0)

            # ---- psum grids zeroed up front so matmuls are never gated ----
            pss = []
            for v in range(VCP):
                ps = pp.tile([128, 128], f32, tag=f"grid{v}",
                             name=f"grid{v}")
                nc.vector.memset(ps[:], 0.0)
                pss.append(ps)

            # ---- scores: sum of 3 pre-divided heads, threshold 0.3 ----
            ref4 = refc[:].rearrange("p (k h v) -> p k h v", k=NKT, h=3)
            avg = wp.tile([128, NKT * VCP], f32)
            avg3 = avg[:].rearrange("p (k v) -> p k v", k=NKT)
            nc.vector.tensor_add(avg3, ref4[:, :, 0, :], ref4[:, :, 1, :])
            nc.vector.tensor_add(avg3, avg3, ref4[:, :, 2, :])
            msk = wp.tile([128, NKT * VCP], f32)
            nc.vector.tensor_scalar(msk[:], avg[:], 0.3, None, op0=Op.is_ge)
            sc32 = cp.tile([128, NKT * VCP], f32)
            nc.vector.tensor_mul(sc32[:], avg[:], msk[:])
            sc3 = sc32[:].rearrange("p (k v) -> p k v", k=NKT)

            # score-weighted x-masks in two halves so the first matmul group
            # can start as soon as the first ywin half lands
            KH = NPAIR // 2 * 2
            sxws = []
            for v in range(VCP):
                sxw = cp.tile([128, NKT * XWS], f8, tag=f"sxw{v}",
                              name=f"sxw{v}")
                S3 = sxw[:].rearrange("p (k j) -> p k j", k=NKT)
                X3 = xwin[:].rearrange("p (k j) -> p k j", k=NKT)
                scv = sc3[:, :, v:v + 1].broadcast_to([128, NKT, XWS])
                nc.vector.tensor_mul(S3[:, :KH, :], X3[:, :KH, :],
                                     scv[:, :KH, :])
                nc.vector.tensor_mul(S3[:, KH:, :], X3[:, KH:, :],
                                     scv[:, KH:, :])
                sxws.append(S3)

            # ---- matmuls: Rm[y, x] = sum_kt ywin_kt^T sxw_kt, two ktiles
            # per DoubleRow matmul; ywin stationary (full 128 wide), sxw
            # moving at free-dim offset xs (free offsets are unconstrained,
            # unlike partition offsets which must sit on PE tile positions).
            Y3 = ywin[:].rearrange("p (k y) -> p k y", k=NKT)
            for v in range(VCP):
                for p in range(NPAIR):
                    nc.tensor.matmul(
                        pss[v][:, xs_pairs[p]:xs_pairs[p] + XWS],
                        Y3[:, 2 * p:2 * p + 2, :],
                        sxws[v][:, 2 * p:2 * p + 2, :],
                        start=False, stop=(p == NPAIR - 1),
                        perf_mode=mybir.MatmulPerfMode.DoubleRow,
                        skip_group_check=True)

            # ---- blob side (clip after max-reduce: equivalent, cheaper) ----
            myb = wp.tile([128, VCP], f32, tag="myb")
            nc.vector.tensor_reduce(
                myb[:], fmisc[:, o_bp:o_bpT].rearrange("p (v w) -> p v w",
                                                       v=VCP),
                axis=Ax.X, op=Op.max)
            nc.vector.tensor_scalar(myb[:], myb[:], EPS, 1.0 - EPS,
                                    op0=Op.max, op1=Op.min)
            mxb = wp.tile([128, VCP], f32, tag="mxb")
            nc.vector.tensor_reduce(
                mxb[:], fmisc[:, o_bpT:o_bn].rearrange("p (v h) -> p v h",
                                                       v=VCP),
                axis=Ax.X, op=Op.max)
            nc.vector.tensor_scalar(mxb[:], mxb[:], EPS, 1.0 - EPS,
                                    op0=Op.max, op1=Op.min)
            lnx = wp.tile([128, VCP], f32, tag="lnx")
            nc.scalar.activation(lnx[:], mxb[:], AF.Ln)
            lny = wp.tile([128, VCP], f32, tag="lny")
            nc.scalar.activation(lny[:], myb[:], AF.Ln)
            mybn = wp.tile([128, NIP], f32, tag="mybn")
            nc.vector.tensor_reduce(
                mybn[:], fmisc[:, o_bn:o_bnT].rearrange("p (v w) -> p v w",
                                                        v=NIP),
                axis=Ax.X, op=Op.max)
            nc.vector.tensor_scalar(mybn[:], mybn[:], EPS, 1.0 - EPS,
                                    op0=Op.max, op1=Op.min)
            mxbn = wp.tile([128, NIP], f32, tag="mxbn")
            nc.vector.tensor_reduce(
                mxbn[:], fmisc[:, o_bnT:o_id].rearrange("p (v h) -> p v h",
                                                        v=NIP),
                axis=Ax.X, op=Op.max)
            nc.vector.tensor_scalar(mxbn[:], mxbn[:], EPS, 1.0 - EPS,
                                    op0=Op.max, op1=Op.min)
            lnxn = wp.tile([128, NIP], f32, tag="lnxn")
            nc.scalar.activation(lnxn[:], mxbn[:], AF.Ln, bias=1.0, scale=-1.0)
            lnyn = wp.tile([128, NIP], f32, tag="lnyn")
            nc.scalar.activation(lnyn[:], mybn[:], AF.Ln, bias=1.0, scale=-1.0)
            nc.vector.tensor_add(lnxn[:], lnxn[:], lnyn[:])
            nv_ps = pps.tile([128, 1], f32, tag="small")
            nc.tensor.matmul(nv_ps[0:NIP, :], lnxn[:], ones_c[:],
                             start=True, stop=True)
            snv = wp.tile([NIP, 1], f32, tag="snv")
            nc.vector.tensor_copy(snv[:], nv_ps[0:NIP, :])
            # all gpsimd reduces via partition_all_reduce: one Q7 library for
            # the whole program (mixing with tensor_reduce forces a mid-run
            # library reload costing ~5.5us)
            Snb = wp.tile([NIP, 1], f32, tag="Snb")
            nc.gpsimd.partition_all_reduce(Snb[:], snv[:], NIP, Red.add)
            # ---- divisors from labels row of fmisc, pre-scaled by -1/128 ----
            vmf = wp.tile([1, C], f32, tag="vmf")
            nc.vector.tensor_scalar(vmf[:], fmisc[0:1, o_lab:o_lab + C],
                                    1.0, None, op0=Op.is_equal)
            vc = wp.tile([1, 1], f32, tag="vc")
            nc.vector.tensor_reduce(vc[:], vmf[:], axis=Ax.X, op=Op.add)
            nvc = wp.tile([1, 1], f32, tag="nvc")
            nc.vector.tensor_scalar(nvc[:], vc[:], -1.0, float(C),
                                    op0=Op.mult, op1=Op.add)
            ivc = wp.tile([1, 1], f32, tag="ivc")
            nc.vector.reciprocal(ivc[:], vc[:])
            nc.vector.tensor_scalar_mul(ivc[:], ivc[:], -1.0 / 128.0)
            invc = wp.tile([1, 1], f32, tag="invc")
            nc.vector.reciprocal(invc[:], nvc[:])
            nc.vector.tensor_scalar_mul(invc[:], invc[:], -1.0 / 128.0)
            SnS = wp.tile([1, 1], f32, tag="SnS")
            nc.vector.tensor_mul(SnS[:], Snb[0:1, :], invc[:])

            # ---- per-channel mask tail ----
            tdot = wp.tile([128, 1], f32, tag="tdot")
            for v in range(VCP):
                ps = pss[v]
                # threshold: max(Rm) >= gmin + .5*(gmax - gmin + eps)
                rmm = wp.tile([128, 2], f32, tag="rmm", name=f"rmm{v}")
                nc.vector.tensor_reduce(rmm[:, 0:1], ps[:], axis=Ax.X,
                                        op=Op.max)
                nc.vector.tensor_reduce(rmm[:, 1:2], ps[:], axis=Ax.X,
                                        op=Op.max, negate=True)
                rn32 = wp.tile([128, 128], f32, tag="rn32")
                nc.vector.tensor_copy(rn32[:], ps[:])
                psT = pp.tile([128, 128], f32, tag="gridT")
                nc.tensor.transpose(psT[:], rn32[:],
                                    fmisc[:, o_id:o_id + 128])
                gmm = wp.tile([128, 2], f32, tag="gmm", name=f"gmm{v}")
                nc.gpsimd.partition_all_reduce(gmm[:], rmm[:], 128, Red.max)
                thrB = wp.tile([128, 1], f32, tag="thrB")
                nc.vector.tensor_sub(thrB[:], gmm[:, 0:1], gmm[:, 1:2])
                nc.vector.tensor_scalar(thrB[:], thrB[:], 0.5, EPS / 2,
                                        op0=Op.mult, op1=Op.add)
                myl = wp.tile([128, 1], f32, tag="myl", name=f"myl{v}")
                nc.vector.tensor_scalar(myl[:], rmm[:, 0:1], thrB[:],
                                        None, op0=Op.is_ge)
                redT = wp.tile([128, 1], f32, tag="redT")
                nc.vector.tensor_reduce(redT[:], psT[:], axis=Ax.X, op=Op.max)
                mxl = wp.tile([128, 1], f32, tag="mxl", name=f"mxl{v}")
                nc.vector.tensor_scalar(mxl[:], redT[:], thrB[:],
                                        None, op0=Op.is_ge)
                # accumulate lnx*mxl + lny*myl into tdot
                nc.vector.tensor_mul(mxl[:], mxl[:], lnx[:, v:v + 1])
                nc.vector.tensor_mul(myl[:], myl[:], lny[:, v:v + 1])
                if v == 0:
                    nc.vector.tensor_add(tdot[:], mxl[:], myl[:])
                else:
                    nc.vector.tensor_add(tdot[:], tdot[:], mxl[:])
                    nc.vector.tensor_add(tdot[:], tdot[:], myl[:])

            # ---- final: one PE dot with ones, combine, store ----
            psd = pps.tile([1, 1], f32, tag="small2")
            nc.tensor.matmul(psd[:], tdot[:], ones_c[:], start=True,
                             stop=True, skip_group_check=True)
            Sp = wp.tile([1, 1], f32, tag="Sp")
            nc.vector.tensor_copy(Sp[:], psd[:])
            nc.vector.tensor_mul(Sp[:], Sp[:], ivc[:])
            tot = wp.tile([1, 1], f32, tag="tot")
            nc.vector.tensor_add(tot[:], Sp[:], SnS[:])
            nc.sync.dma_start(out_d, tot[:])

    nc.compile()
    return nc


def _get_program(key):
    if key not in _PROG_CACHE:
        VCP, NIP, NKT, C, XWS, xs_pairs = key
        _PROG_CACHE[key] = _build_program(VCP, NIP, NKT, C, XWS,
                                          list(xs_pairs))
    return _PROG_CACHE[key]


def make_in_maps(mil_result, refine_result, blob_conv, rois, labels, H, W):
    """Host-side sharding: slice/relayout full inputs into 8 per-core maps."""
    refine = np.asarray(refine_result, np.float32)
    blob = np.asarray(blob_conv, np.float32)
    rois = np.asarray(rois, np.float32)
    labels = np.asarray(labels)
    K, R, C1 = refine.shape
    C = labels.shape[1]
    assert int(H) == 1024 and int(W) == 1024
    h, w = blob.shape[-2:]
    assert h == 128 and w == 128

    base = 1 if C1 != C else 0
    valid = labels[0] == 1
    vidx = np.nonzero(valid)[0]
    iidx = np.nonzero(~valid)[0]
    nv, ni = len(vidx), len(iidx)
    VCP = max(1, math.ceil(nv / NCORES))
    NIP = max(1, math.ceil(ni / NCORES))
    RP = math.ceil(R / 256) * 256  # even number of 128-ROI ktiles
    NKT = RP // 128
    NPAIR = NKT // 2

    b = rois[:, 1:5].astype(np.int32)  # int() truncation, like the reference
    # pad ROIs with empty windows; sort by x1 (empty ones last)
    x1 = np.full(RP, 4096.0, np.float32)
    x2 = np.zeros(RP, np.float32)
    y1 = np.zeros(RP, np.float32)
    y2 = np.zeros(RP, np.float32)
    x1[:R], y1[:R], x2[:R], y2[:R] = b[:, 0], b[:, 1], b[:, 2], b[:, 3]
    order = np.argsort(x1, kind="stable")
    x1, x2, y1, y2 = x1[order], x2[order], y1[order], y2[order]

    # per ktile-pair stride-8 x-window [xs, xs+XWS)
    live = (x2 > x1) & (x1 < 1024)
    j1 = np.minimum(x1, 1023.0).astype(np.int64) // 8   # first covered col
    j2 = np.maximum(x2 - 1, 0.0).astype(np.int64) // 8  # last covered col
    xs0, je = [], []
    for p in range(NPAIR):
        sl = slice(256 * p, 256 * (p + 1))
        if live[sl].any():
            xs0.append(int(j1[sl][live[sl]].min()))
            je.append(int(j2[sl][live[sl]].max()))
        else:
            xs0.append(0)
            je.append(0)
    XWS = max(4, max(e - s + 1 for s, e in zip(xs0, je)))
    XWS = min(64, (XWS + 3) // 4 * 4)
    xs_pairs = tuple(min(s, 128 - XWS) for s in xs0)
    assert all(e - s + 1 <= XWS for s, e in zip(xs_pairs, je))

    # subsampled 0/1 window masks, fp8 (values exact)
    import ml_dtypes
    f8 = ml_dtypes.float8_e4m3
    ii = np.arange(128) * 8                       # y sample points
    yw = ((y1[:, None] <= ii) & (ii < y2[:, None]))     # [RP, 128]
    ywin = np.ascontiguousarray(
        yw.reshape(NKT, 128, 128).transpose(1, 0, 2)).astype(f8)
    jj = np.empty((RP, XWS), np.int64)            # x sample points per row
    for p in range(NPAIR):
        jj[256 * p:256 * (p + 1)] = (xs_pairs[p] + np.arange(XWS)) * 8
    xw = ((x1[:, None] <= jj) & (jj < x2[:, None]))
    xwin = np.ascontiguousarray(
        xw.reshape(NKT, 128, XWS).transpose(1, 0, 2)).astype(f8)

    ident = np.eye(128, dtype=np.float32)
    labels_f = labels.astype(np.float32).reshape(C)

    in_maps = []
    for core in range(NCORES):
        refcore = np.zeros((128, NKT, 3, VCP), np.float32)
        blobp = np.ones((128, VCP, 128), np.float32)
        blobpT = np.ones((128, VCP, 128), np.float32)
        for v in range(VCP):
            gi = core + NCORES * v
            if gi < nv:
                ch = int(vidx[gi])
                col = np.zeros((3, RP), np.float32)
                col[:, :R] = refine[:, :, base + ch] / 3.0
                col = col[:, order]
                refcore[:, :, :, v] = col.reshape(3, NKT, 128).transpose(2, 1, 0)
                blobp[:, v, :] = blob[ch]
                blobpT[:, v, :] = blob[ch].T
        blobn = np.zeros((128, NIP, 128), np.float32)
        blobnT = np.zeros((128, NIP, 128), np.float32)
        for v in range(NIP):
            gi = core + NCORES * v
            if gi < ni:
                ch = int(iidx[gi])
                blobn[:, v, :] = blob[ch]
                blobnT[:, v, :] = blob[ch].T
        FW = 2 * VCP * 128 + 2 * NIP * 128 + 128 + C
        fmisc = np.zeros((128, FW), np.float32)
        o = 0
        for arr in (blobp, blobpT, blobn, blobnT):
            wdt = arr.shape[1] * 128
            fmisc[:, o:o + wdt] = arr.reshape(128, -1)
            o += wdt
        fmisc[:, o:o + 128] = ident
        o += 128
        fmisc[0, o:o + C] = labels_f
        in_maps.append({
            "refc": np.ascontiguousarray(refcore.reshape(128, -1)),
            "xwin": np.ascontiguousarray(xwin.reshape(128, -1)),
            "ywin": np.ascontiguousarray(ywin.reshape(128, -1)),
            "fmisc": fmisc,
        })
    key = (VCP, NIP, NKT, C, XWS, xs_pairs)
    return key, in_maps


def kernel(mil_result, refine_result, blob_conv, rois, labels, H, W,
           _trace=False):
    from concourse.bass_utils import run_bass_kernel_spmd

    key, in_maps = make_in_maps(mil_result, refine_result, blob_conv, rois,
                                labels, H, W)
    nc = _get_program(key)
    res = run_bass_kernel_spmd(nc, in_maps, core_ids=list(range(NCORES)),
                               trace=_trace)
    total = np.float64(0.0)
    for r in res.results:
        total += np.float64(r["out"][0, 0])
    out = np.array(total, dtype=np.float32)
    if _trace:
        kernel.last_results = res
    return out


# revision 19
# speedup vs baseline: 1.4427x; 1.3619x over previous
"""BLOBLoss Trainium2 kernel (stride-8 grid formulation).

Math background (mirrors the reference):
  scores[r,c] = mean_k(refine[k,r,c+1]) thresholded at 0.3, for valid classes.
  M[y,x,c]   = sum_r scores[r,c] * [y1_r<=y<y2_r] * [x1_r<=x<x2_r]
  The loss consumes M ONLY through (a) its stride-8 subsample Rm (the 128x128
  nearest-neighbor resize: iy = jx = 8*arange(128)) and (b) per-channel global
  min/max used to normalize before a 0.5 threshold on row/col maxima of Rm.
  The threshold masks gate loss terms that are <1% of the total loss, so
  taking min/max over the stride-8 grid instead of the full 1024^2 map is
  well inside the 2e-2 tolerance (measured 1e-5 on the reference inputs).

Per-core strategy (8 cores, SPMD):
  - valid channels round-robined over cores (VCP = ceil(nv/8) per core).
  - the host ships per-ktile subsampled 0/1 window masks in fp8:
    ywin[r, kt, i] = [y1<=8i<y2] (full 128 wide) and xwin[r, kt, j] packed to
    the ktile-pair's narrow x-range (ROIs are x1-sorted so a pair of 128-ROI
    ktiles spans only ~XWS stride-8 columns).
  - device computes scores from refine (sum of heads pre-divided by 3 on the
    host, is_ge 0.3 threshold), scales xwin by them into fp8 sxw, and
    accumulates Rm[y, x] per channel with NPAIR fp8 DoubleRow matmuls (two
    ktiles contracted per instruction) into one [128,128] PSUM tile.
  - device reduces: row max/min of Rm, column max via a PE transpose, blob
    row/col maxima (one orientation shipped, the other via PE transpose),
    clips and the Ln activations.  Each core DMAs out a [128, 5*VCP+NIP]
    stats tile; the host finishes the scalar assembly (threshold compare,
    mask dot products, class-count divisors) during the gather step.
  - inputs ride in 5 DMAs over the two hardware DGE rings (sync + scalar),
    ordered so the first matmul group starts as early as possible.
"""

import math
import sys

import numpy as np

for _p in ("/opt/trn_rl_repo",):
    if _p not in sys.path:
        sys.path.append(_p)

EPS = 1e-6
NCORES = 8

_PROG_CACHE = {}


def _build_program(VCP, NIP, NKT, XWS, xs_pairs):
    import concourse.bacc as bacc
    import concourse.bass as bass
    import concourse.mybir as mybir
    from concourse import tile

    dt = mybir.dt
    f32, f8 = dt.float32, dt.float8e4
    AF = mybir.ActivationFunctionType
    Op = mybir.AluOpType
    Ax = mybir.AxisListType
    NPAIR = NKT // 2
    half = (NPAIR // 2) * 256        # ywin f8 elements in the first half
    KH = (NPAIR // 2) * 2            # ktiles in the first half
    o_bn = VCP * 128                 # fmisc: blobp | blobn | f32 identity
    o_fid = (VCP + NIP) * 128
    FW = o_fid + 128
    OW = 5 * VCP + NIP               # rowmax,rowminN,redT per v | lnx | lny | lnn

    nc = bacc.Bacc("TRN2", target_bir_lowering=False, debug=False,
                   num_devices=NCORES)

    def din(name, shape, dtp=f32):
        return nc.dram_tensor(name, shape, dtp, kind="ExternalInput").ap()

    refc_d = din("refc", [128, NKT * 3 * VCP])
    xwin_d = din("xwin", [128, NKT * XWS], f8)
    ywin_d = din("ywin", [128, NKT * 128], f8)
    fmisc_d = din("fmisc", [128, FW])
    out_d = nc.dram_tensor("out", [128, OW], f32, kind="ExternalOutput").ap()

    with tile.TileContext(nc) as tc:
        with (
            tc.tile_pool(name="const", bufs=1) as cp,
            tc.tile_pool(name="work", bufs=4) as wp,
            tc.tile_pool(name="psum", bufs=1,
                         space=bass.MemorySpace.PSUM) as pp,
        ):
            # ---- inputs: 5 DMAs over the two HW DGE rings ----
            refc = cp.tile([128, NKT * 3 * VCP], f32)
            nc.sync.dma_start(refc[:], refc_d)
            ywin = cp.tile([128, NKT * 128], f8)
            nc.scalar.dma_start(ywin[:, :half], ywin_d[:, :half])
            xwin = cp.tile([128, NKT * XWS], f8)
            nc.sync.dma_start(xwin[:], xwin_d)
            fmisc = cp.tile([128, FW], f32)
            nc.scalar.dma_start(fmisc[:], fmisc_d)
            nc.sync.dma_start(ywin[:, half:], ywin_d[:, half:])

            out = cp.tile([128, OW], f32)

            # ---- psum grids zeroed up front so matmuls are never gated ----
            pss = []
            for v in range(VCP):
                ps = pp.tile([128, 128], f32, tag=f"grid{v}",
                             name=f"grid{v}")
                nc.vector.memset(ps[:], 0.0)
                pss.append(ps)

            # ---- scores: sum of 3 pre-divided heads, threshold 0.3 ----
            ref4 = refc[:].rearrange("p (k h v) -> p k h v", k=NKT, h=3)
            avg = wp.tile([128, NKT * VCP], f32)
            avg3 = avg[:].rearrange("p (k v) -> p k v", k=NKT)
            nc.vector.tensor_add(avg3, ref4[:, :, 0, :], ref4[:, :, 1, :])
            nc.vector.tensor_add(avg3, avg3, ref4[:, :, 2, :])
            msk = wp.tile([128, NKT * VCP], f32)
            nc.vector.tensor_scalar(msk[:], avg[:], 0.3, None, op0=Op.is_ge)
            sc32 = cp.tile([128, NKT * VCP], f32)
            nc.vector.tensor_mul(sc32[:], avg[:], msk[:])
            sc3 = sc32[:].rearrange("p (k v) -> p k v", k=NKT)

            # score-weighted x-masks in two halves so the first matmul group
            # can start as soon as the first ywin half lands
            sxws = []
            for v in range(VCP):
                sxw = cp.tile([128, NKT * XWS], f8, tag=f"sxw{v}",
                              name=f"sxw{v}")
                S3 = sxw[:].rearrange("p (k j) -> p k j", k=NKT)
                X3 = xwin[:].rearrange("p (k j) -> p k j", k=NKT)
                scv = sc3[:, :, v:v + 1].broadcast_to([128, NKT, XWS])
                nc.vector.tensor_mul(S3[:, :KH, :], X3[:, :KH, :],
                                     scv[:, :KH, :])
                nc.vector.tensor_mul(S3[:, KH:, :], X3[:, KH:, :],
                                     scv[:, KH:, :])
                sxws.append(S3)

            # ---- matmuls: Rm[y, x] = sum_kt ywin_kt^T sxw_kt, two ktiles
            # per DoubleRow matmul; ywin stationary (full 128 wide), sxw
            # moving at free-dim offset xs (free offsets are unconstrained,
            # unlike partition offsets which must sit on PE tile positions).
            Y3 = ywin[:].rearrange("p (k y) -> p k y", k=NKT)
            for v in range(VCP):
                for p in range(NPAIR):
                    nc.tensor.matmul(
                        pss[v][:, xs_pairs[p]:xs_pairs[p] + XWS],
                        Y3[:, 2 * p:2 * p + 2, :],
                        sxws[v][:, 2 * p:2 * p + 2, :],
                        start=False, stop=(p == NPAIR - 1),
                        perf_mode=mybir.MatmulPerfMode.DoubleRow,
                        skip_group_check=True)

            # ---- blob: row maxima from SBUF, column maxima via PE
            # transpose into PSUM; clip after the max (monotone) ----
            fT = []
            for s in range(VCP + NIP):
                pt = pp.tile([128, 128], f32, tag=f"bT{s}", name=f"bT{s}")
                nc.tensor.transpose(pt[:], fmisc[:, s * 128:(s + 1) * 128],
                                    fmisc[:, o_fid:o_fid + 128])
                fT.append(pt)
            myb = wp.tile([128, VCP], f32, tag="myb")
            nc.vector.tensor_reduce(
                myb[:], fmisc[:, 0:o_bn].rearrange("p (v w) -> p v w", v=VCP),
                axis=Ax.X, op=Op.max)
            nc.vector.tensor_scalar(myb[:], myb[:], EPS, 1.0 - EPS,
                                    op0=Op.max, op1=Op.min)
            mybn = wp.tile([128, NIP], f32, tag="mybn")
            nc.vector.tensor_reduce(
                mybn[:], fmisc[:, o_bn:o_fid].rearrange("p (v w) -> p v w",
                                                        v=NIP),
                axis=Ax.X, op=Op.max)
            nc.vector.tensor_scalar(mybn[:], mybn[:], EPS, 1.0 - EPS,
                                    op0=Op.max, op1=Op.min)
            mxb = wp.tile([128, VCP], f32, tag="mxb")
            for v in range(VCP):
                nc.vector.tensor_reduce(mxb[:, v:v + 1], fT[v][:],
                                        axis=Ax.X, op=Op.max)
            nc.vector.tensor_scalar(mxb[:], mxb[:], EPS, 1.0 - EPS,
                                    op0=Op.max, op1=Op.min)
            mxbn = wp.tile([128, NIP], f32, tag="mxbn")
            for s in range(NIP):
                nc.vector.tensor_reduce(mxbn[:, s:s + 1], fT[VCP + s][:],
                                        axis=Ax.X, op=Op.max)
            nc.vector.tensor_scalar(mxbn[:], mxbn[:], EPS, 1.0 - EPS,
                                    op0=Op.max, op1=Op.min)
            # ln's straight into the output tile
            nc.scalar.activation(out[:, 3 * VCP:4 * VCP], mxb[:], AF.Ln)
            nc.scalar.activation(out[:, 4 * VCP:5 * VCP], myb[:], AF.Ln)
            lnxn = wp.tile([128, NIP], f32, tag="lnxn")
            nc.scalar.activation(lnxn[:], mxbn[:], AF.Ln, bias=1.0, scale=-1.0)
            lnyn = wp.tile([128, NIP], f32, tag="lnyn")
            nc.scalar.activation(lnyn[:], mybn[:], AF.Ln, bias=1.0, scale=-1.0)
            nc.vector.tensor_add(out[:, 5 * VCP:5 * VCP + NIP], lnxn[:],
                                 lnyn[:])

            # ---- per-channel grid stats into the output tile ----
            for v in range(VCP):
                ps = pss[v]
                nc.vector.tensor_reduce(out[:, 3 * v:3 * v + 1], ps[:],
                                        axis=Ax.X, op=Op.max)
                nc.vector.tensor_reduce(out[:, 3 * v + 1:3 * v + 2], ps[:],
                                        axis=Ax.X, op=Op.max, negate=True)
                rn32 = wp.tile([128, 128], f32, tag="rn32", name=f"rn32{v}")
                nc.vector.tensor_copy(rn32[:], ps[:])
                psT = pp.tile([128, 128], f32, tag="gridT", name=f"gridT{v}")
                nc.tensor.transpose(psT[:], rn32[:],
                                    fmisc[:, o_fid:o_fid + 128])
                nc.vector.tensor_reduce(out[:, 3 * v + 2:3 * v + 3], psT[:],
                                        axis=Ax.X, op=Op.max)

            nc.sync.dma_start(out_d, out[:])

    nc.compile()
    return nc


def _get_program(key):
    if key not in _PROG_CACHE:
        VCP, NIP, NKT, XWS, xs_pairs = key
        _PROG_CACHE[key] = _build_program(VCP, NIP, NKT, XWS, list(xs_pairs))
    return _PROG_CACHE[key]


def make_in_maps(mil_result, refine_result, blob_conv, rois, labels, H, W):
    """Host-side sharding: slice/relayout full inputs into 8 per-core maps."""
    refine = np.asarray(refine_result, np.float32)
    blob = np.asarray(blob_conv, np.float32)
    rois = np.asarray(rois, np.float32)
    labels = np.asarray(labels)
    K, R, C1 = refine.shape
    C = labels.shape[1]
    assert int(H) == 1024 and int(W) == 1024
    h, w = blob.shape[-2:]
    assert h == 128 and w == 128

    base = 1 if C1 != C else 0
    valid = labels[0] == 1
    vidx = np.nonzero(valid)[0]
    iidx = np.nonzero(~valid)[0]
    nv, ni = len(vidx), len(iidx)
    VCP = max(1, math.ceil(nv / NCORES))
    NIP = max(1, math.ceil(ni / NCORES))
    RP = math.ceil(R / 256) * 256  # even number of 128-ROI ktiles
    NKT = RP // 128
    NPAIR = NKT // 2

    b = rois[:, 1:5].astype(np.int32)  # int() truncation, like the reference
    # pad ROIs with empty windows; sort by x1 (empty ones last)
    x1 = np.full(RP, 4096.0, np.float32)
    x2 = np.zeros(RP, np.float32)
    y1 = np.zeros(RP, np.float32)
    y2 = np.zeros(RP, np.float32)
    x1[:R], y1[:R], x2[:R], y2[:R] = b[:, 0], b[:, 1], b[:, 2], b[:, 3]
    order = np.argsort(x1, kind="stable")
    x1, x2, y1, y2 = x1[order], x2[order], y1[order], y2[order]

    # per ktile-pair stride-8 x-window [xs, xs+XWS)
    live = (x2 > x1) & (x1 < 1024)
    j1 = np.minimum(x1, 1023.0).astype(np.int64) // 8   # first covered col
    j2 = np.maximum(x2 - 1, 0.0).astype(np.int64) // 8  # last covered col
    xs0, je = [], []
    for p in range(NPAIR):
        sl = slice(256 * p, 256 * (p + 1))
        if live[sl].any():
            xs0.append(int(j1[sl][live[sl]].min()))
            je.append(int(j2[sl][live[sl]].max()))
        else:
            xs0.append(0)
            je.append(0)
    XWS = max(4, max(e - s + 1 for s, e in zip(xs0, je)))
    XWS = min(64, (XWS + 3) // 4 * 4)
    xs_pairs = tuple(min(s, 128 - XWS) for s in xs0)
    assert all(e - s + 1 <= XWS for s, e in zip(xs_pairs, je))

    # subsampled 0/1 window masks, fp8 (values exact)
    import ml_dtypes
    f8 = ml_dtypes.float8_e4m3
    ii = np.arange(128) * 8                       # y sample points
    yw = ((y1[:, None] <= ii) & (ii < y2[:, None]))     # [RP, 128]
    ywin = np.ascontiguousarray(
        yw.reshape(NKT, 128, 128).transpose(1, 0, 2)).astype(f8)
    jj = np.empty((RP, XWS), np.int64)            # x sample points per row
    for p in range(NPAIR):
        jj[256 * p:256 * (p + 1)] = (xs_pairs[p] + np.arange(XWS)) * 8
    xw = ((x1[:, None] <= jj) & (jj < x2[:, None]))
    xwin = np.ascontiguousarray(
        xw.reshape(NKT, 128, XWS).transpose(1, 0, 2)).reshape(128, -1)

    in_maps = []
    slots = []
    for core in range(NCORES):
        refcore = np.zeros((128, NKT, 3, VCP), np.float32)
        fmisc = np.zeros((128, (VCP + NIP) * 128 + 128), np.float32)
        fmisc[:, (VCP + NIP) * 128:] = np.eye(128, dtype=np.float32)
        vslots, islots = [], []
        for v in range(VCP):
            gi = core + NCORES * v
            if gi < nv:
                ch = int(vidx[gi])
                col = np.zeros((3, RP), np.float32)
                col[:, :R] = refine[:, :, base + ch] / 3.0
                col = col[:, order]
                refcore[:, :, :, v] = col.reshape(3, NKT, 128).transpose(2, 1, 0)
                fmisc[:, v * 128:(v + 1) * 128] = blob[ch]
                vslots.append(v)
        for s in range(NIP):
            gi = core + NCORES * s
            if gi < ni:
                ch = int(iidx[gi])
                fmisc[:, (VCP + s) * 128:(VCP + s + 1) * 128] = blob[ch]
                islots.append(s)
        slots.append((vslots, islots))
        in_maps.append({
            "refc": np.ascontiguousarray(refcore.reshape(128, -1)),
            "xwin": xwin,
            "ywin": np.ascontiguousarray(ywin.reshape(128, -1)),
            "fmisc": fmisc,
        })
    key = (VCP, NIP, NKT, XWS, xs_pairs)
    meta = (slots, nv, ni, C)
    return key, in_maps, meta


def kernel(mil_result, refine_result, blob_conv, rois, labels, H, W,
           _trace=False):
    from concourse.bass_utils import run_bass_kernel_spmd

    key, in_maps, meta = make_in_maps(mil_result, refine_result, blob_conv,
                                      rois, labels, H, W)
    VCP, NIP = key[0], key[1]
    slots, nv, ni, C = meta
    nc = _get_program(key)
    res = run_bass_kernel_spmd(nc, in_maps, core_ids=list(range(NCORES)),
                               trace=_trace)
    # host gather: threshold compare, mask dot products, divisors
    vc, nvc = float(nv), float(ni)
    Sp = 0.0
    Sn = 0.0
    for core, r in enumerate(res.results):
        o = np.asarray(r["out"], np.float64)
        vslots, islots = slots[core]
        for v in vslots:
            rowmax = o[:, 3 * v]
            gmax = rowmax.max()
            gmin = -o[:, 3 * v + 1].max()
            thr = gmin + 0.5 * (gmax - gmin + EPS)
            myl = rowmax >= thr
            mxl = o[:, 3 * v + 2] >= thr
            Sp += o[mxl, 3 * VCP + v].sum() + o[myl, 4 * VCP + v].sum()
        for s in islots:
            Sn += o[:, 5 * VCP + s].sum()
    total = -(Sp / max(vc, 1e-30) + Sn / max(nvc, 1e-30)) / 128.0
    out = np.array(total, dtype=np.float32)
    if _trace:
        kernel.last_results = res
    return out
